# revision 5
# baseline (speedup 1.0000x reference)
"""Trainium2 Bass kernel for GRU + ragged unpad + L2 normalize.

Problem: B=16, T=2048, D=H=1024 single-layer GRU (torch gate order r,z,n),
then per-sequence unpad to flat [sum(lengths), H] and L2-normalize rows.

Strategy: the sequential scan is PE-weight-load bound (192 [128x128] weight
tiles per step, free dim only 2).  Instead run block-parallel Picard
iteration: for a block of S timesteps, iterate
    H^{k+1}_t = GRUStep(H^k_{t-1}, xg_t)   for all t in the block at once,
which turns the recurrence into sweeps of one large GEMM (N = S columns,
weight loads amortized) + elementwise gates.  The GRU map is contractive
(z-gate), so sweep error decays geometrically.  9 sweeps (the first is
GEMM-free since the initial guess is zero) reach rel err ~1.2e-2
end-to-end, validated offline vs the reference with matching arithmetic.
Blocks chain sequentially via the carry h.

The sweep GEMM runs in fp8 DoubleRow perf mode (256-row contraction per
matmul): w_hh and a shadow copy of H are fp8e4m3; the master H stays bf16
for the elementwise update and the final normalize.  Elementwise work is
spread over DVE / ACT / GPSIMD so the PE stays the bottleneck.

Sharding: data-parallel over batch, 2 sequences per core across 8 cores.
"""

import numpy as np
import ml_dtypes

B, T, D = 16, 2048, 1024
G3 = 3 * D           # 3072 gate rows
NCORES = 8
BPC = B // NCORES    # 2 sequences per core
KC = D // 128        # 8 contraction chunks
MC = G3 // 128       # 24 gate chunks (r: 0..7, z: 8..15, n: 16..23)
HC = D // 128        # 8 hidden chunks
SB = 512             # Picard block length (timesteps)
HP = SB + 8          # padded H columns (stride alignment for DoubleRow)
GSWEEPS = 8          # GEMM sweeps per block (after the free zero-guess sweep)
TA = 512             # phase A token block
EPS = 1e-12

_cache = {}


def _blocks_for(t2):
    out = []
    t = 0
    while t < t2:
        out.append(min(SB, t2 - t))
        t += SB
    return out


def _build(t2: int):
    """Build the per-core Bass kernel covering t2 timesteps."""
    import concourse.mybir as mybir
    import concourse.tile as tile
    from concourse import bacc

    f32 = mybir.dt.float32
    bf16 = mybir.dt.bfloat16
    fp8 = mybir.dt.float8e4
    AF = mybir.ActivationFunctionType
    DR = mybir.MatmulPerfMode.DoubleRow

    blocks = _blocks_for(t2)
    assert GSWEEPS % 2 == 0

    nc = bacc.Bacc("TRN2", enable_partition_id=False)

    xT = nc.dram_tensor("xT", [KC, 128, BPC * t2], bf16, kind="ExternalInput")
    wihT = nc.dram_tensor("wihT", [KC, 128, G3], bf16, kind="ExternalInput")
    whhT = nc.dram_tensor("whhT", [KC, 128, G3], fp8, kind="ExternalInput")
    bih = nc.dram_tensor("bih", [128, MC], f32, kind="ExternalInput")
    bhh = nc.dram_tensor("bhh", [128, MC], f32, kind="ExternalInput")
    yout = nc.dram_tensor("yout", [HC, 128, BPC * t2], f32, kind="ExternalOutput")
    xg_d = nc.dram_tensor("xg_d", [128, MC, BPC, t2], bf16, kind="Internal")

    with tile.TileContext(nc) as tc:
        with tc.tile_pool(name="persist", bufs=1) as pp:
            whh_sb = pp.tile([128, KC, G3], fp8, tag="whh")
            bih_sb = pp.tile([128, MC], f32, tag="bih")
            bhh_sb = pp.tile([128, MC], f32, tag="bhh")
            ones_k = pp.tile([128, 1], bf16, tag="ones_k")
            ones_m = pp.tile([1, 128], f32, tag="ones_m")
            zt = pp.tile([128, SB], bf16, tag="zt")
            hcar = pp.tile([128, KC, BPC], bf16, tag="hcar")

            for k in range(KC):
                nc.sync.dma_start(out=whh_sb[:, k, :], in_=whhT[k, :, :])
            nc.sync.dma_start(out=bih_sb, in_=bih[:, :])
            nc.sync.dma_start(out=bhh_sb, in_=bhh[:, :])
            nc.vector.memset(ones_k, 1.0)
            nc.vector.memset(ones_m, 1.0)
            nc.vector.memset(zt, 0.0)
            nc.vector.memset(hcar, 0.0)

            # ---------------- Phase A: xg = x @ w_ih.T + b_ih (bf16 out) ----
            with (
                tc.tile_pool(name="pa_w", bufs=1) as paw,
                tc.tile_pool(name="pa_x", bufs=3) as pax,
                tc.tile_pool(name="pa_o", bufs=4) as pao,
                tc.tile_pool(name="pa_ps", bufs=2, space="PSUM") as paps,
            ):
                wih_sb = paw.tile([128, KC, G3], bf16, tag="wih")
                for k in range(KC):
                    nc.sync.dma_start(out=wih_sb[:, k, :], in_=wihT[k, :, :])
                for b in range(BPC):
                    for t0 in range(0, t2, TA):
                        tn = min(TA, t2 - t0)
                        xa = pax.tile([128, KC, TA], bf16, tag="xa")
                        for k in range(KC):
                            nc.sync.dma_start(
                                out=xa[:, k, :tn],
                                in_=xT[k, :, b * t2 + t0 : b * t2 + t0 + tn],
                            )
                        for m in range(MC):
                            ps = paps.tile([128, TA], f32, tag="ps")
                            for k in range(KC):
                                nc.tensor.matmul(
                                    ps[:, :tn],
                                    wih_sb[:, k, m * 128 : (m + 1) * 128],
                                    xa[:, k, :tn],
                                    start=(k == 0),
                                    stop=(k == KC - 1),
                                )
                            xo = pao.tile([128, TA], bf16, tag="xo")
                            nc.scalar.activation(
                                xo[:, :tn], ps[:, :tn], AF.Identity,
                                bias=bih_sb[:, m : m + 1],
                            )
                            nc.sync.dma_start(
                                out=xg_d[:, m, b, t0 : t0 + tn],
                                in_=xo[:, :tn],
                            )

            # ---------------- Picard blocks ----------------
            with (
                tc.tile_pool(name="pb_xg", bufs=1) as pbx,
                tc.tile_pool(name="pb_h", bufs=1) as pbh,
                tc.tile_pool(name="pb_g", bufs=3) as pbg,
                tc.tile_pool(name="pb_o", bufs=3) as pbo,
                tc.tile_pool(name="pb_r", bufs=2, space="PSUM") as psr,
                tc.tile_pool(name="pb_z", bufs=2, space="PSUM") as psz,
                tc.tile_pool(name="pb_n", bufs=2, space="PSUM") as psn,
                tc.tile_pool(name="pc_s", bufs=1, space="PSUM") as pcs,
                tc.tile_pool(name="pc_b", bufs=1, space="PSUM") as pcb,
            ):
                xg_sb = pbx.tile([128, MC, BPC, SB], bf16, tag="xg")
                H0 = pbh.tile([128, KC, BPC, HP], bf16, tag="h0")
                H1 = pbh.tile([128, KC, BPC, HP], bf16, tag="h1")
                F0 = pbh.tile([128, KC, BPC, HP], fp8, tag="f0")
                F1 = pbh.tile([128, KC, BPC, HP], fp8, tag="f1")

                def gates(j, s, S, pr, pz, pn, Hr, Hw, Fw):
                    """Common gate elementwise chain. pr/pz/pn are psum APs
                    (or xg slices for the GEMM-free sweep, signalled by
                    pr is None)."""
                    if pr is None:
                        rin = xg_sb[:, j, s, :S]
                        zin = xg_sb[:, HC + j, s, :S]
                        hnin = zt[:, :S]
                    else:
                        tr = pbg.tile([128, SB], bf16, tag="tr")
                        nc.vector.tensor_add(
                            tr[:, :S], pr[:, :S], xg_sb[:, j, s, :S]
                        )
                        rin = tr[:, :S]
                        tz = pbg.tile([128, SB], bf16, tag="tz")
                        nc.vector.tensor_add(
                            tz[:, :S], pz[:, :S], xg_sb[:, HC + j, s, :S]
                        )
                        zin = tz[:, :S]
                        hnin = pn[:, :S]
                    r = pbg.tile([128, SB], bf16, tag="r")
                    nc.scalar.activation(
                        r[:, :S], rin, AF.Sigmoid, bias=bhh_sb[:, j : j + 1]
                    )
                    z = pbg.tile([128, SB], bf16, tag="z")
                    nc.scalar.activation(
                        z[:, :S], zin, AF.Sigmoid,
                        bias=bhh_sb[:, HC + j : HC + j + 1],
                    )
                    hn = pbg.tile([128, SB], bf16, tag="hn")
                    nc.scalar.activation(
                        hn[:, :S], hnin, AF.Identity,
                        bias=bhh_sb[:, 2 * HC + j : 2 * HC + j + 1],
                    )
                    t_ = pbg.tile([128, SB], bf16, tag="t")
                    nc.vector.tensor_mul(t_[:, :S], r[:, :S], hn[:, :S])
                    nc.vector.tensor_add(
                        t_[:, :S], t_[:, :S], xg_sb[:, 2 * HC + j, s, :S]
                    )
                    n_ = pbg.tile([128, SB], bf16, tag="n")
                    nc.scalar.activation(n_[:, :S], t_[:, :S], AF.Tanh)
                    d_ = pbg.tile([128, SB], bf16, tag="d")
                    nc.gpsimd.tensor_sub(d_[:, :S], Hr[:, j, s, :S], n_[:, :S])
                    nc.gpsimd.tensor_mul(d_[:, :S], z[:, :S], d_[:, :S])
                    nc.vector.tensor_add(
                        Hw[:, j, s, 1 : S + 1], n_[:, :S], d_[:, :S]
                    )
                    nc.gpsimd.tensor_add(
                        Fw[:, j, s, 1 : S + 1], n_[:, :S], d_[:, :S]
                    )

                def sweep0(Hr, Hw, Fw, S):
                    for j in range(HC):
                        for s in range(BPC):
                            gates(j, s, S, None, None, None, Hr, Hw, Fw)

                def sweep(Fr, Hr, Hw, Fw, S):
                    for j in range(HC):
                        for s in range(BPC):
                            pr = psr.tile([128, SB], f32, tag="pr")
                            pz = psz.tile([128, SB], f32, tag="pz")
                            pn = psn.tile([128, SB], f32, tag="pn")
                            for g, ps in ((0, pr), (1, pz), (2, pn)):
                                m = g * HC + j
                                for kp in range(KC // 2):
                                    nc.tensor.matmul(
                                        ps[:, :S],
                                        whh_sb[:, 2 * kp : 2 * kp + 2,
                                               m * 128 : (m + 1) * 128],
                                        Fr[:, 2 * kp : 2 * kp + 2, s, :S],
                                        start=(kp == 0),
                                        stop=(kp == KC // 2 - 1),
                                        perf_mode=DR,
                                    )
                            gates(j, s, S, pr, pz, pn, Hr, Hw, Fw)

                nc.vector.memset(H1, 0.0)
                nc.vector.memset(F0, 0.0)
                nc.vector.memset(F1, 0.0)

                t0 = 0
                for bi, S in enumerate(blocks):
                    nc.vector.memset(H0, 0.0)
                    if bi > 0:
                        for s in range(BPC):
                            for Hx in (H0, H1, F0, F1):
                                nc.vector.tensor_copy(
                                    Hx[:, :, s, 0], hcar[:, :, s]
                                )
                    for mg in range(4):
                        m0, m1 = mg * (MC // 4), (mg + 1) * (MC // 4)
                        nc.sync.dma_start(
                            out=xg_sb[:, m0:m1, :, :S],
                            in_=xg_d[:, m0:m1, :, t0 : t0 + S],
                        )
                    sweep0(H0, H1, F1, S)
                    for _ in range(GSWEEPS // 2):
                        sweep(F1, H1, H0, F0, S)
                        sweep(F0, H0, H1, F1, S)
                    # final result is in H1
                    if bi + 1 < len(blocks):
                        for s in range(BPC):
                            nc.vector.tensor_copy(hcar[:, :, s], H1[:, :, s, S])

                    # ---------- Phase C: L2 normalize + store ----------
                    for s in range(BPC):
                        pss = pcs.tile([1, SB], f32, tag="pss")
                        for j in range(HC):
                            sq = pbg.tile([128, SB], bf16, tag="sq")
                            nc.vector.tensor_mul(
                                sq[:, :S], H1[:, j, s, 1 : S + 1],
                                H1[:, j, s, 1 : S + 1],
                            )
                            nc.tensor.matmul(
                                pss[:, :S], ones_k, sq[:, :S],
                                start=(j == 0), stop=(j == HC - 1),
                            )
                        nrm = pbg.tile([1, SB], f32, tag="nrm")
                        nc.scalar.activation(nrm[:, :S], pss[:, :S], AF.Sqrt)
                        nc.vector.tensor_scalar_max(nrm[:, :S], nrm[:, :S], EPS)
                        rs = pbg.tile([1, SB], f32, tag="rs")
                        nc.vector.reciprocal(rs[:, :S], nrm[:, :S])
                        psb = pcb.tile([128, SB], f32, tag="psb")
                        nc.tensor.matmul(
                            psb[:, :S], ones_m, rs[:, :S], start=True, stop=True
                        )
                        for j in range(HC):
                            ysc = pbo.tile([128, SB], f32, tag="ysc")
                            nc.vector.tensor_mul(
                                ysc[:, :S], H1[:, j, s, 1 : S + 1], psb[:, :S]
                            )
                            nc.sync.dma_start(
                                out=yout[j, :, s * t2 + t0 : s * t2 + t0 + S],
                                in_=ysc[:, :S],
                            )
                    t0 += S

    nc.compile()
    return nc


def _build_noop():
    """Same I/O signature as _build but a trivial body — used by test.py to
    subtract dispatch/transfer overhead from wall-clock timing."""
    import concourse.mybir as mybir
    import concourse.tile as tile
    from concourse import bacc

    f32 = mybir.dt.float32
    bf16 = mybir.dt.bfloat16
    fp8 = mybir.dt.float8e4
    t2 = _t2_cached[0]
    nc = bacc.Bacc("TRN2", enable_partition_id=False)
    nc.dram_tensor("xT", [KC, 128, BPC * t2], bf16, kind="ExternalInput")
    nc.dram_tensor("wihT", [KC, 128, G3], bf16, kind="ExternalInput")
    nc.dram_tensor("whhT", [KC, 128, G3], fp8, kind="ExternalInput")
    bih = nc.dram_tensor("bih", [128, MC], f32, kind="ExternalInput")
    nc.dram_tensor("bhh", [128, MC], f32, kind="ExternalInput")
    yout = nc.dram_tensor("yout", [HC, 128, BPC * t2], f32, kind="ExternalOutput")
    with tile.TileContext(nc) as tc:
        with tc.tile_pool(name="p", bufs=1) as p:
            t = p.tile([128, MC], f32, tag="t")
            nc.sync.dma_start(out=t, in_=bih[:, :])
            nc.sync.dma_start(out=yout[0, :, :MC], in_=t)
    nc.compile()
    return nc


_t2_cached = [2048]


def _prep_inputs(x, w_ih, w_hh, b_ih, b_hh, t2):
    """Host-side layout prep (not timed): transposes + dtype casts."""
    bf = ml_dtypes.bfloat16
    f8 = ml_dtypes.float8_e4m3
    x = np.asarray(x, dtype=np.float32)[:, :t2]
    wihT = np.ascontiguousarray(np.asarray(w_ih, np.float32).T).astype(bf)
    whhT = np.ascontiguousarray(np.asarray(w_hh, np.float32).T).astype(f8)
    wihT = wihT.reshape(KC, 128, G3)
    whhT = whhT.reshape(KC, 128, G3)
    bih = np.ascontiguousarray(
        np.asarray(b_ih, np.float32).reshape(MC, 128).T
    )
    bhh = np.ascontiguousarray(
        np.asarray(b_hh, np.float32).reshape(MC, 128).T
    )
    in_maps = []
    for c in range(NCORES):
        xc = x[c * BPC : (c + 1) * BPC]            # [2, t2, D]
        xTc = np.ascontiguousarray(xc.transpose(2, 0, 1))  # [D, 2, t2]
        xTc = xTc.reshape(KC, 128, BPC * t2).astype(bf)
        in_maps.append(
            {"xT": xTc, "wihT": wihT, "whhT": whhT, "bih": bih, "bhh": bhh}
        )
    return in_maps


def _assemble(results, lengths, t2):
    """Per-core yout [HC,128,BPC*t2] fp32 -> flat [sum(lengths), D]."""
    lengths = np.asarray(lengths).astype(np.int64)
    parts = []
    for c in range(NCORES):
        yo = np.asarray(results[c]["yout"], np.float32)
        yo = yo.reshape(D, BPC, t2).transpose(1, 2, 0)  # [2, t2, D]
        for b in range(BPC):
            parts.append(yo[b, : lengths[c * BPC + b]])
    return np.concatenate(parts, axis=0)


def kernel(x, lengths, w_ih, w_hh, b_ih, b_hh):
    from concourse import bass_utils

    lengths_np = np.asarray(lengths).astype(np.int64)
    max_len = int(lengths_np.max())
    t2 = min(T, -(-max_len // 16) * 16)
    _t2_cached[0] = t2
    if t2 not in _cache:
        _cache[t2] = _build(t2)
    nc = _cache[t2]

    in_maps = _prep_inputs(x, w_ih, w_hh, b_ih, b_hh, t2)
    res = bass_utils.run_bass_kernel_spmd(nc, in_maps, list(range(NCORES)))
    return _assemble(res.results, lengths_np, t2)


if __name__ == "__main__":
    import reference

    inputs = reference.setup_inputs()
    out = kernel(**{k: np.asarray(v) for k, v in inputs.items()})
    exp = np.asarray(reference.reference(**inputs))
    err = np.abs(out - exp).max()
    rel = np.linalg.norm(out - exp) / np.linalg.norm(exp)
    print("absmax:", err, "rel:", rel)


# revision 18
# speedup vs baseline: 2.2245x; 2.2245x over previous
"""Trainium2 Bass kernel for GRU + ragged unpad + L2 normalize.

Problem: B=16, T=2048, D=H=1024 single-layer GRU (torch gate order r,z,n),
then per-sequence unpad to flat [sum(lengths), H] and L2-normalize rows.

Strategy: the sequential scan is PE-weight-load bound (192 [128x128] weight
tiles per step, free dim only 2).  Instead run block-parallel Picard
iteration: for a block of S timesteps, iterate
    H^{k+1}_t = GRUStep(H^k_{t-1}, xg_t)   for all t in the block at once,
which turns the recurrence into sweeps of one large GEMM (N = S columns,
weight loads amortized) + elementwise gates.  The GRU map is contractive
(z-gate), so sweep error decays geometrically.  9 sweeps (the first is
GEMM-free since the initial guess is zero) reach rel err ~1.2e-2
end-to-end, validated offline vs the reference with matching arithmetic.
Blocks chain sequentially via the carry h.

The sweep GEMM runs in fp8 DoubleRow perf mode (256-row contraction per
matmul): w_hh and a shadow copy of H are fp8e4m3; the master H stays bf16
for the elementwise update and the final normalize.  Elementwise work is
spread over DVE / ACT / GPSIMD so the PE stays the bottleneck.

Sharding: data-parallel over batch, 2 sequences per core across 8 cores.
"""

import os
import numpy as np
import ml_dtypes

B, T, D = 16, 2048, 1024
G3 = 3 * D           # 3072 gate rows
NCORES = 8
BPC = B // NCORES    # 2 sequences per core
KC = D // 128        # 8 contraction chunks
MC = G3 // 128       # 24 gate chunks (r: 0..7, z: 8..15, n: 16..23)
HC = D // 128        # 8 hidden chunks
SB = 512             # Picard block length (timesteps)
HP = SB + 8          # padded H columns (stride alignment for DoubleRow)
GSWEEPS = int(os.environ.get("GSW", "5"))  # full GEMM sweeps per block (after the free zero-guess sweep)
NONLY = int(os.environ.get("NONLY", "3"))  # n-gate-only sweeps (r/z frozen)
TA = 512             # phase A token block
EPS = 1e-12

_cache = {}


def _blocks_for(t2):
    out = []
    t = 0
    while t < t2:
        out.append(min(SB, t2 - t))
        t += SB
    return out


def _build(t2: int):
    """Build the per-core Bass kernel covering t2 timesteps."""
    import concourse.mybir as mybir
    import concourse.tile as tile
    from concourse import bacc

    f32 = mybir.dt.float32
    bf16 = mybir.dt.bfloat16
    fp8 = mybir.dt.float8e4
    AF = mybir.ActivationFunctionType
    DR = mybir.MatmulPerfMode.DoubleRow

    blocks = _blocks_for(t2)

    nc = bacc.Bacc("TRN2", enable_partition_id=False)

    xT = nc.dram_tensor("xT", [KC, 128, BPC * t2], bf16, kind="ExternalInput")
    wihT = nc.dram_tensor("wihT", [KC, 128, G3], bf16, kind="ExternalInput")
    whhT = nc.dram_tensor("whhT", [KC, 128, G3], fp8, kind="ExternalInput")
    bih = nc.dram_tensor("bih", [128, MC], f32, kind="ExternalInput")
    bhh = nc.dram_tensor("bhh", [128, MC], f32, kind="ExternalInput")
    eye = nc.dram_tensor("eye", [128, 128], bf16, kind="ExternalInput")
    bhhT = nc.dram_tensor("bhhT", [1, G3], bf16, kind="ExternalInput")
    yout = nc.dram_tensor("yout", [HC, 128, BPC * t2], f32, kind="ExternalOutput")
    xg_d = nc.dram_tensor("xg_d", [128, MC, BPC, t2], bf16, kind="Internal")

    with tile.TileContext(nc) as tc:
        with tc.tile_pool(name="persist", bufs=1) as pp:
            whh_sb = pp.tile([128, KC, G3], fp8, tag="whh")
            bih_sb = pp.tile([128, MC], f32, tag="bih")
            bhh_sb = pp.tile([128, MC], f32, tag="bhh")
            ones_k = pp.tile([128, 1], bf16, tag="ones_k")
            ones_m = pp.tile([1, 128], f32, tag="ones_m")
            ones_r = pp.tile([1, SB], bf16, tag="ones_r")
            eye_sb = pp.tile([128, 128], bf16, tag="eye")
            bhhT_sb = pp.tile([1, G3], bf16, tag="bhhT")
            zt = pp.tile([128, SB], bf16, tag="zt")
            hcar = pp.tile([128, KC, BPC], bf16, tag="hcar")

            for k in range(KC):
                nc.sync.dma_start(out=whh_sb[:, k, :], in_=whhT[k, :, :])
            nc.sync.dma_start(out=bih_sb, in_=bih[:, :])
            nc.sync.dma_start(out=bhh_sb, in_=bhh[:, :])
            nc.sync.dma_start(out=eye_sb, in_=eye[:, :])
            nc.sync.dma_start(out=bhhT_sb, in_=bhhT[:, :])
            nc.vector.memset(ones_k, 1.0)
            nc.vector.memset(ones_m, 1.0)
            nc.vector.memset(ones_r, 1.0)
            nc.vector.memset(zt, 0.0)
            nc.vector.memset(hcar, 0.0)

            # ---------------- Phase A: xg = x @ w_ih.T + b_ih (bf16 out) ----
            with (
                tc.tile_pool(name="pa_w", bufs=1) as paw,
                tc.tile_pool(name="pa_x", bufs=3) as pax,
                tc.tile_pool(name="pa_o", bufs=4) as pao,
                tc.tile_pool(name="pa_ps", bufs=2, space="PSUM") as paps,
            ):
                wih_sb = paw.tile([128, KC, G3], bf16, tag="wih")
                for k in range(KC):
                    nc.sync.dma_start(out=wih_sb[:, k, :], in_=wihT[k, :, :])
                for b in range(BPC):
                    for t0 in range(0, t2, TA):
                        tn = min(TA, t2 - t0)
                        xa = pax.tile([128, KC, TA], bf16, tag="xa")
                        for k in range(KC):
                            nc.sync.dma_start(
                                out=xa[:, k, :tn],
                                in_=xT[k, :, b * t2 + t0 : b * t2 + t0 + tn],
                            )
                        for m in range(MC):
                            ps = paps.tile([128, TA], f32, tag="ps")
                            for k in range(KC):
                                nc.tensor.matmul(
                                    ps[:, :tn],
                                    wih_sb[:, k, m * 128 : (m + 1) * 128],
                                    xa[:, k, :tn],
                                    start=(k == 0),
                                    stop=(k == KC - 1),
                                )
                            xo = pao.tile([128, TA], bf16, tag="xo")
                            nc.scalar.activation(
                                xo[:, :tn], ps[:, :tn], AF.Identity,
                                bias=bih_sb[:, m : m + 1],
                            )
                            nc.sync.dma_start(
                                out=xg_d[:, m, b, t0 : t0 + tn],
                                in_=xo[:, :tn],
                            )

            # ---------------- Picard blocks ----------------
            with (
                tc.tile_pool(name="pb_xg", bufs=1) as pbx,
                tc.tile_pool(name="pb_h", bufs=1) as pbh,
                tc.tile_pool(name="pb_g", bufs=3) as pbg,
                tc.tile_pool(name="pb_o", bufs=3) as pbo,
                tc.tile_pool(name="pb_r", bufs=2, space="PSUM") as psr,
                tc.tile_pool(name="pb_z", bufs=2, space="PSUM") as psz,
                tc.tile_pool(name="pb_n", bufs=2, space="PSUM") as psn,
                tc.tile_pool(name="pc_s", bufs=1, space="PSUM") as pcs,
                tc.tile_pool(name="pc_b", bufs=1, space="PSUM") as pcb,
            ):
                xg_sb = pbx.tile([128, MC, BPC, SB], bf16, tag="xg")
                H0 = pbh.tile([128, KC, BPC, HP], bf16, tag="h0")
                H1 = pbh.tile([128, KC, BPC, HP], bf16, tag="h1")
                F0 = pbh.tile([128, KC, BPC, HP], fp8, tag="f0")
                F1 = pbh.tile([128, KC, BPC, HP], fp8, tag="f1")
                rc = pbh.tile([128, HC, BPC, SB], bf16, tag="rc")
                zc = pbh.tile([128, HC, BPC, SB], bf16, tag="zc")

                def tail(j, s, S, pn_or_hn, Hr, Hw, Fw):
                    """n-gate + h-update chain shared by all sweep kinds.
                    Reads r/z from the rc/zc caches."""
                    t_ = pbg.tile([128, SB], bf16, tag="t")
                    nc.vector.tensor_mul(t_[:, :S], rc[:, j, s, :S], pn_or_hn)
                    nc.vector.tensor_add(
                        t_[:, :S], t_[:, :S], xg_sb[:, 2 * HC + j, s, :S]
                    )
                    n_ = pbg.tile([128, SB], bf16, tag="n")
                    nc.scalar.activation(n_[:, :S], t_[:, :S], AF.Tanh)
                    d_ = pbg.tile([128, SB], bf16, tag="d")
                    nc.vector.tensor_sub(d_[:, :S], Hr[:, j, s, :S], n_[:, :S])
                    nc.vector.tensor_mul(d_[:, :S], zc[:, j, s, :S], d_[:, :S])
                    nc.vector.tensor_add(
                        Hw[:, j, s, 1 : S + 1], n_[:, :S], d_[:, :S]
                    )
                    nc.gpsimd.tensor_add(
                        Fw[:, j, s, 1 : S + 1], n_[:, :S], d_[:, :S]
                    )

                def sweep0(Hr, Hw, Fw, S):
                    for j in range(HC):
                        for s in range(BPC):
                            nc.scalar.activation(
                                rc[:, j, s, :S], xg_sb[:, j, s, :S],
                                AF.Sigmoid, bias=bhh_sb[:, j : j + 1],
                            )
                            nc.scalar.activation(
                                zc[:, j, s, :S], xg_sb[:, HC + j, s, :S],
                                AF.Sigmoid,
                                bias=bhh_sb[:, HC + j : HC + j + 1],
                            )
                            hn = pbg.tile([128, SB], bf16, tag="hn")
                            nc.scalar.activation(
                                hn[:, :S], zt[:, :S], AF.Identity,
                                bias=bhh_sb[:, 2 * HC + j : 2 * HC + j + 1],
                            )
                            tail(j, s, S, hn[:, :S], Hr, Hw, Fw)

                def sweep(Fr, Hr, Hw, Fw, S, full):
                    for j in range(HC):
                        for s in range(BPC):
                            if full:
                                pr = psr.tile([128, SB], f32, tag="pr")
                                pz = psz.tile([128, SB], f32, tag="pz")
                                # seed r/z psums with xg (identity MM); whh
                                # accumulates on top in fp8 DoubleRow mode.
                                nc.tensor.matmul(
                                    pr[:, :S], eye_sb, xg_sb[:, j, s, :S],
                                    start=True, stop=False,
                                )
                                nc.tensor.matmul(
                                    pz[:, :S], eye_sb, xg_sb[:, HC + j, s, :S],
                                    start=True, stop=False,
                                )
                            pn = psn.tile([128, SB], f32, tag="pn")
                            m_n = 2 * HC + j
                            nc.tensor.matmul(
                                pn[:, :S],
                                bhhT_sb[0:1, m_n * 128 : (m_n + 1) * 128],
                                ones_r[0:1, :S],
                                start=True, stop=False,
                            )
                            groups = (
                                ((0, pr), (1, pz), (2, pn)) if full
                                else ((2, pn),)
                            )
                            for g, ps in groups:
                                m = g * HC + j
                                for kp in range(KC // 2):
                                    nc.tensor.matmul(
                                        ps[:, :S],
                                        whh_sb[:, 2 * kp : 2 * kp + 2,
                                               m * 128 : (m + 1) * 128],
                                        Fr[:, 2 * kp : 2 * kp + 2, s, :S],
                                        start=False,
                                        stop=(kp == KC // 2 - 1),
                                        perf_mode=DR,
                                    )
                            if full:
                                nc.scalar.activation(
                                    rc[:, j, s, :S], pr[:, :S], AF.Sigmoid,
                                    bias=bhh_sb[:, j : j + 1],
                                )
                                nc.scalar.activation(
                                    zc[:, j, s, :S], pz[:, :S], AF.Sigmoid,
                                    bias=bhh_sb[:, HC + j : HC + j + 1],
                                )
                            tail(j, s, S, pn[:, :S], Hr, Hw, Fw)

                nc.vector.memset(H1, 0.0)
                nc.vector.memset(F0, 0.0)
                nc.vector.memset(F1, 0.0)

                t0 = 0
                for bi, S in enumerate(blocks):
                    nc.vector.memset(H0, 0.0)
                    if bi > 0:
                        for s in range(BPC):
                            for Hx in (H0, H1, F0, F1):
                                nc.vector.tensor_copy(
                                    Hx[:, :, s, 0], hcar[:, :, s]
                                )
                    for mg in range(4):
                        m0, m1 = mg * (MC // 4), (mg + 1) * (MC // 4)
                        nc.sync.dma_start(
                            out=xg_sb[:, m0:m1, :, :S],
                            in_=xg_d[:, m0:m1, :, t0 : t0 + S],
                        )
                    sweep0(H0, H1, F1, S)
                    assert (GSWEEPS + NONLY) % 2 == 0
                    bufs = [(F1, H1, H0, F0), (F0, H0, H1, F1)]
                    for i in range(GSWEEPS + NONLY):
                        sweep(*bufs[i % 2], S, full=(i < GSWEEPS))
                    # final result is in H1
                    if bi + 1 < len(blocks):
                        for s in range(BPC):
                            nc.vector.tensor_copy(hcar[:, :, s], H1[:, :, s, S])

                    # ---------- Phase C: L2 normalize + store ----------
                    for s in range(BPC):
                        pss = pcs.tile([1, SB], f32, tag="pss")
                        for j in range(HC):
                            sq = pbg.tile([128, SB], bf16, tag="sq")
                            nc.vector.tensor_mul(
                                sq[:, :S], H1[:, j, s, 1 : S + 1],
                                H1[:, j, s, 1 : S + 1],
                            )
                            nc.tensor.matmul(
                                pss[:, :S], ones_k, sq[:, :S],
                                start=(j == 0), stop=(j == HC - 1),
                            )
                        nrm = pbg.tile([1, SB], f32, tag="nrm")
                        nc.scalar.activation(nrm[:, :S], pss[:, :S], AF.Sqrt)
                        nc.vector.tensor_scalar_max(nrm[:, :S], nrm[:, :S], EPS)
                        rs = pbg.tile([1, SB], f32, tag="rs")
                        nc.vector.reciprocal(rs[:, :S], nrm[:, :S])
                        psb = pcb.tile([128, SB], f32, tag="psb")
                        nc.tensor.matmul(
                            psb[:, :S], ones_m, rs[:, :S], start=True, stop=True
                        )
                        for j in range(HC):
                            ysc = pbo.tile([128, SB], f32, tag="ysc")
                            nc.vector.tensor_mul(
                                ysc[:, :S], H1[:, j, s, 1 : S + 1], psb[:, :S]
                            )
                            nc.sync.dma_start(
                                out=yout[j, :, s * t2 + t0 : s * t2 + t0 + S],
                                in_=ysc[:, :S],
                            )
                    t0 += S

    nc.compile()
    return nc


def _build_noop():
    """Same I/O signature as _build but a trivial body — used by test.py to
    subtract dispatch/transfer overhead from wall-clock timing."""
    import concourse.mybir as mybir
    import concourse.tile as tile
    from concourse import bacc

    f32 = mybir.dt.float32
    bf16 = mybir.dt.bfloat16
    fp8 = mybir.dt.float8e4
    t2 = _t2_cached[0]
    nc = bacc.Bacc("TRN2", enable_partition_id=False)
    nc.dram_tensor("xT", [KC, 128, BPC * t2], bf16, kind="ExternalInput")
    nc.dram_tensor("wihT", [KC, 128, G3], bf16, kind="ExternalInput")
    nc.dram_tensor("whhT", [KC, 128, G3], fp8, kind="ExternalInput")
    bih = nc.dram_tensor("bih", [128, MC], f32, kind="ExternalInput")
    nc.dram_tensor("bhh", [128, MC], f32, kind="ExternalInput")
    nc.dram_tensor("eye", [128, 128], bf16, kind="ExternalInput")
    nc.dram_tensor("bhhT", [1, G3], bf16, kind="ExternalInput")
    yout = nc.dram_tensor("yout", [HC, 128, BPC * t2], f32, kind="ExternalOutput")
    with tile.TileContext(nc) as tc:
        with tc.tile_pool(name="p", bufs=1) as p:
            t = p.tile([128, MC], f32, tag="t")
            nc.sync.dma_start(out=t, in_=bih[:, :])
            nc.sync.dma_start(out=yout[0, :, :MC], in_=t)
    nc.compile()
    return nc


_t2_cached = [2048]


def _prep_inputs(x, w_ih, w_hh, b_ih, b_hh, t2):
    """Host-side layout prep (not timed): transposes + dtype casts."""
    bf = ml_dtypes.bfloat16
    f8 = ml_dtypes.float8_e4m3
    x = np.asarray(x, dtype=np.float32)[:, :t2]
    wihT = np.ascontiguousarray(np.asarray(w_ih, np.float32).T).astype(bf)
    whhT = np.ascontiguousarray(np.asarray(w_hh, np.float32).T).astype(f8)
    wihT = wihT.reshape(KC, 128, G3)
    whhT = whhT.reshape(KC, 128, G3)
    bih = np.ascontiguousarray(
        np.asarray(b_ih, np.float32).reshape(MC, 128).T
    )
    bhh = np.ascontiguousarray(
        np.asarray(b_hh, np.float32).reshape(MC, 128).T
    )
    eye = np.eye(128, dtype=bf)
    bhhT = np.asarray(b_hh, np.float32).reshape(1, G3).astype(bf)
    in_maps = []
    for c in range(NCORES):
        xc = x[c * BPC : (c + 1) * BPC]            # [2, t2, D]
        xTc = np.ascontiguousarray(xc.transpose(2, 0, 1))  # [D, 2, t2]
        xTc = xTc.reshape(KC, 128, BPC * t2).astype(bf)
        in_maps.append(
            {"xT": xTc, "wihT": wihT, "whhT": whhT, "bih": bih, "bhh": bhh,
             "eye": eye, "bhhT": bhhT}
        )
    return in_maps


def _assemble(results, lengths, t2):
    """Per-core yout [HC,128,BPC*t2] fp32 -> flat [sum(lengths), D]."""
    lengths = np.asarray(lengths).astype(np.int64)
    parts = []
    for c in range(NCORES):
        yo = np.asarray(results[c]["yout"], np.float32)
        yo = yo.reshape(D, BPC, t2).transpose(1, 2, 0)  # [2, t2, D]
        for b in range(BPC):
            parts.append(yo[b, : lengths[c * BPC + b]])
    return np.concatenate(parts, axis=0)


def kernel(x, lengths, w_ih, w_hh, b_ih, b_hh):
    from concourse import bass_utils

    lengths_np = np.asarray(lengths).astype(np.int64)
    max_len = int(lengths_np.max())
    t2 = min(T, -(-max_len // 16) * 16)
    _t2_cached[0] = t2
    if t2 not in _cache:
        _cache[t2] = _build(t2)
    nc = _cache[t2]

    in_maps = _prep_inputs(x, w_ih, w_hh, b_ih, b_hh, t2)
    res = bass_utils.run_bass_kernel_spmd(nc, in_maps, list(range(NCORES)))
    return _assemble(res.results, lengths_np, t2)


if __name__ == "__main__":
    import reference

    inputs = reference.setup_inputs()
    out = kernel(**{k: np.asarray(v) for k, v in inputs.items()})
    exp = np.asarray(reference.reference(**inputs))
    err = np.abs(out - exp).max()
    rel = np.linalg.norm(out - exp) / np.linalg.norm(exp)
    print("absmax:", err, "rel:", rel)


# revision 19
# speedup vs baseline: 11.2310x; 5.0488x over previous
"""Trainium2 Bass kernel for GRU + ragged unpad + L2 normalize.

Problem: B=16, T=2048, D=H=1024 single-layer GRU (torch gate order r,z,n),
then per-sequence unpad to flat [sum(lengths), H] and L2-normalize rows.

Strategy: the sequential scan is PE-weight-load bound (192 [128x128] weight
tiles per step, free dim only 2).  Instead run block-parallel Picard
iteration: for a block of S timesteps, iterate
    H^{k+1}_t = GRUStep(H^k_{t-1}, xg_t)   for all t in the block at once,
which turns the recurrence into sweeps of one large GEMM (N = S columns,
weight loads amortized) + elementwise gates.  The GRU map is contractive
(z-gate), so sweep error decays geometrically.  9 sweeps (the first is
GEMM-free since the initial guess is zero) reach rel err ~1.2e-2
end-to-end, validated offline vs the reference with matching arithmetic.
Blocks chain sequentially via the carry h.

The sweep GEMM runs in fp8 DoubleRow perf mode (256-row contraction per
matmul): w_hh and a shadow copy of H are fp8e4m3; the master H stays bf16
for the elementwise update and the final normalize.  Elementwise work is
spread over DVE / ACT / GPSIMD so the PE stays the bottleneck.

Sharding: data-parallel over batch, 2 sequences per core across 8 cores.
"""

import os
import numpy as np
import ml_dtypes

B, T, D = 16, 2048, 1024
G3 = 3 * D           # 3072 gate rows
NCORES = 8
BPC = B // NCORES    # 2 sequences per core
KC = D // 128        # 8 contraction chunks
MC = G3 // 128       # 24 gate chunks (r: 0..7, z: 8..15, n: 16..23)
HC = D // 128        # 8 hidden chunks
SB = 512             # Picard block length (timesteps)
HP = SB + 8          # padded H columns (stride alignment for DoubleRow)
GSWEEPS = int(os.environ.get("GSW", "4"))  # full GEMM sweeps per block (after the free zero-guess sweep)
NONLY = int(os.environ.get("NONLY", "4"))  # n-gate-only sweeps (r/z frozen)
TA = 512             # phase A token block
EPS = 1e-12

_cache = {}


def _blocks_for(t2):
    out = []
    t = 0
    while t < t2:
        out.append(min(SB, t2 - t))
        t += SB
    return out


def _build(t2: int):
    """Build the per-core Bass kernel covering t2 timesteps."""
    import concourse.mybir as mybir
    import concourse.tile as tile
    from concourse import bacc

    f32 = mybir.dt.float32
    bf16 = mybir.dt.bfloat16
    fp8 = mybir.dt.float8e4
    AF = mybir.ActivationFunctionType
    DR = mybir.MatmulPerfMode.DoubleRow

    blocks = _blocks_for(t2)

    nc = bacc.Bacc("TRN2", enable_partition_id=False)

    xT = nc.dram_tensor("xT", [KC, 128, BPC * t2], bf16, kind="ExternalInput")
    wihT = nc.dram_tensor("wihT", [KC, 128, G3], bf16, kind="ExternalInput")
    whhT = nc.dram_tensor("whhT", [KC, 128, G3], fp8, kind="ExternalInput")
    bih = nc.dram_tensor("bih", [128, MC], f32, kind="ExternalInput")
    bhh = nc.dram_tensor("bhh", [128, MC], f32, kind="ExternalInput")
    eye = nc.dram_tensor("eye", [128, 128], bf16, kind="ExternalInput")
    bhhT = nc.dram_tensor("bhhT", [1, G3], bf16, kind="ExternalInput")
    yout = nc.dram_tensor("yout", [HC, 128, BPC * t2], f32, kind="ExternalOutput")
    xg_d = nc.dram_tensor("xg_d", [128, MC, BPC, t2], bf16, kind="Internal")

    with tile.TileContext(nc) as tc:
        with tc.tile_pool(name="persist", bufs=1) as pp:
            whh_sb = pp.tile([128, KC, G3], fp8, tag="whh")
            bih_sb = pp.tile([128, MC], f32, tag="bih")
            bhh_sb = pp.tile([128, MC], f32, tag="bhh")
            ones_k = pp.tile([128, 1], bf16, tag="ones_k")
            ones_m = pp.tile([1, 128], f32, tag="ones_m")
            ones_r = pp.tile([1, SB], bf16, tag="ones_r")
            eye_sb = pp.tile([128, 128], bf16, tag="eye")
            bhhT_sb = pp.tile([1, G3], bf16, tag="bhhT")
            zt = pp.tile([128, SB], bf16, tag="zt")
            hcar = pp.tile([128, KC, BPC], bf16, tag="hcar")

            for k in range(KC):
                nc.sync.dma_start(out=whh_sb[:, k, :], in_=whhT[k, :, :])
            nc.sync.dma_start(out=bih_sb, in_=bih[:, :])
            nc.sync.dma_start(out=bhh_sb, in_=bhh[:, :])
            nc.sync.dma_start(out=eye_sb, in_=eye[:, :])
            nc.sync.dma_start(out=bhhT_sb, in_=bhhT[:, :])
            nc.vector.memset(ones_k, 1.0)
            nc.vector.memset(ones_m, 1.0)
            nc.vector.memset(ones_r, 1.0)
            nc.vector.memset(zt, 0.0)
            nc.vector.memset(hcar, 0.0)

            # ---------------- Phase A: xg = x @ w_ih.T + b_ih (bf16 out) ----
            with (
                tc.tile_pool(name="pa_w", bufs=1) as paw,
                tc.tile_pool(name="pa_x", bufs=3) as pax,
                tc.tile_pool(name="pa_o", bufs=4) as pao,
                tc.tile_pool(name="pa_ps", bufs=2, space="PSUM") as paps,
            ):
                wih_sb = paw.tile([128, KC, G3], bf16, tag="wih")
                for k in range(KC):
                    nc.sync.dma_start(out=wih_sb[:, k, :], in_=wihT[k, :, :])
                for b in range(BPC):
                    for t0 in range(0, t2, TA):
                        tn = min(TA, t2 - t0)
                        xa = pax.tile([128, KC, TA], bf16, tag="xa")
                        for k in range(KC):
                            nc.sync.dma_start(
                                out=xa[:, k, :tn],
                                in_=xT[k, :, b * t2 + t0 : b * t2 + t0 + tn],
                            )
                        for m in range(MC):
                            ps = paps.tile([128, TA], f32, tag="ps")
                            for k in range(KC):
                                nc.tensor.matmul(
                                    ps[:, :tn],
                                    wih_sb[:, k, m * 128 : (m + 1) * 128],
                                    xa[:, k, :tn],
                                    start=(k == 0),
                                    stop=(k == KC - 1),
                                )
                            xo = pao.tile([128, TA], bf16, tag="xo")
                            nc.scalar.activation(
                                xo[:, :tn], ps[:, :tn], AF.Identity,
                                bias=bih_sb[:, m : m + 1],
                            )
                            nc.sync.dma_start(
                                out=xg_d[:, m, b, t0 : t0 + tn],
                                in_=xo[:, :tn],
                            )

            # ---------------- Picard blocks ----------------
            with (
                tc.tile_pool(name="pb_xg", bufs=1) as pbx,
                tc.tile_pool(name="pb_h", bufs=1) as pbh,
                tc.tile_pool(name="pb_g", bufs=3) as pbg,
                tc.tile_pool(name="pb_o", bufs=3) as pbo,
                tc.tile_pool(name="pb_r", bufs=2, space="PSUM") as psr,
                tc.tile_pool(name="pb_z", bufs=2, space="PSUM") as psz,
                tc.tile_pool(name="pb_n", bufs=2, space="PSUM") as psn,
                tc.tile_pool(name="pc_s", bufs=1, space="PSUM") as pcs,
                tc.tile_pool(name="pc_b", bufs=1, space="PSUM") as pcb,
            ):
                xg_sb = pbx.tile([128, MC, BPC, SB], bf16, tag="xg")
                H0 = pbh.tile([128, KC, BPC, HP], bf16, tag="h0")
                H1 = pbh.tile([128, KC, BPC, HP], bf16, tag="h1")
                F0 = pbh.tile([128, KC, BPC, HP], fp8, tag="f0")
                F1 = pbh.tile([128, KC, BPC, HP], fp8, tag="f1")
                rc = pbh.tile([128, HC, BPC, SB], bf16, tag="rc")
                zc = pbh.tile([128, HC, BPC, SB], bf16, tag="zc")

                def tail(j, s, S, pn_or_hn, Hr, Hw, Fw):
                    """n-gate + h-update chain shared by all sweep kinds.
                    Reads r/z from the rc/zc caches."""
                    t_ = pbg.tile([128, SB], bf16, tag="t")
                    nc.vector.tensor_mul(t_[:, :S], rc[:, j, s, :S], pn_or_hn)
                    nc.vector.tensor_add(
                        t_[:, :S], t_[:, :S], xg_sb[:, 2 * HC + j, s, :S]
                    )
                    n_ = pbg.tile([128, SB], bf16, tag="n")
                    nc.scalar.activation(n_[:, :S], t_[:, :S], AF.Tanh)
                    d_ = pbg.tile([128, SB], bf16, tag="d")
                    nc.vector.tensor_sub(d_[:, :S], Hr[:, j, s, :S], n_[:, :S])
                    nc.vector.tensor_mul(d_[:, :S], zc[:, j, s, :S], d_[:, :S])
                    nc.vector.tensor_add(
                        Hw[:, j, s, 1 : S + 1], n_[:, :S], d_[:, :S]
                    )
                    nc.gpsimd.tensor_add(
                        Fw[:, j, s, 1 : S + 1], n_[:, :S], d_[:, :S]
                    )

                def sweep0(Hr, Hw, Fw, S):
                    for j in range(HC):
                        for s in range(BPC):
                            nc.scalar.activation(
                                rc[:, j, s, :S], xg_sb[:, j, s, :S],
                                AF.Sigmoid, bias=bhh_sb[:, j : j + 1],
                            )
                            nc.scalar.activation(
                                zc[:, j, s, :S], xg_sb[:, HC + j, s, :S],
                                AF.Sigmoid,
                                bias=bhh_sb[:, HC + j : HC + j + 1],
                            )
                            hn = pbg.tile([128, SB], bf16, tag="hn")
                            nc.scalar.activation(
                                hn[:, :S], zt[:, :S], AF.Identity,
                                bias=bhh_sb[:, 2 * HC + j : 2 * HC + j + 1],
                            )
                            tail(j, s, S, hn[:, :S], Hr, Hw, Fw)

                def sweep(Fr, Hr, Hw, Fw, S, full):
                    for j in range(HC):
                        for s in range(BPC):
                            if full:
                                pr = psr.tile([128, SB], f32, tag="pr")
                                pz = psz.tile([128, SB], f32, tag="pz")
                                # seed r/z psums with xg (identity MM); whh
                                # accumulates on top in fp8 DoubleRow mode.
                                nc.tensor.matmul(
                                    pr[:, :S], eye_sb, xg_sb[:, j, s, :S],
                                    start=True, stop=False,
                                )
                                nc.tensor.matmul(
                                    pz[:, :S], eye_sb, xg_sb[:, HC + j, s, :S],
                                    start=True, stop=False,
                                )
                            pn = psn.tile([128, SB], f32, tag="pn")
                            m_n = 2 * HC + j
                            nc.tensor.matmul(
                                pn[:, :S],
                                bhhT_sb[0:1, m_n * 128 : (m_n + 1) * 128],
                                ones_r[0:1, :S],
                                start=True, stop=False,
                            )
                            groups = (
                                ((0, pr), (1, pz), (2, pn)) if full
                                else ((2, pn),)
                            )
                            for g, ps in groups:
                                m = g * HC + j
                                for kp in range(KC // 2):
                                    nc.tensor.matmul(
                                        ps[:, :S],
                                        whh_sb[:, 2 * kp : 2 * kp + 2,
                                               m * 128 : (m + 1) * 128],
                                        Fr[:, 2 * kp : 2 * kp + 2, s, :S],
                                        start=False,
                                        stop=(kp == KC // 2 - 1),
                                        perf_mode=DR,
                                    )
                            if full:
                                nc.scalar.activation(
                                    rc[:, j, s, :S], pr[:, :S], AF.Sigmoid,
                                    bias=bhh_sb[:, j : j + 1],
                                )
                                nc.scalar.activation(
                                    zc[:, j, s, :S], pz[:, :S], AF.Sigmoid,
                                    bias=bhh_sb[:, HC + j : HC + j + 1],
                                )
                            tail(j, s, S, pn[:, :S], Hr, Hw, Fw)

                nc.vector.memset(H1, 0.0)
                nc.vector.memset(F0, 0.0)
                nc.vector.memset(F1, 0.0)

                t0 = 0
                for bi, S in enumerate(blocks):
                    nc.vector.memset(H0, 0.0)
                    if bi > 0:
                        for s in range(BPC):
                            for Hx in (H0, H1, F0, F1):
                                nc.vector.tensor_copy(
                                    Hx[:, :, s, 0], hcar[:, :, s]
                                )
                    for mg in range(4):
                        m0, m1 = mg * (MC // 4), (mg + 1) * (MC // 4)
                        nc.sync.dma_start(
                            out=xg_sb[:, m0:m1, :, :S],
                            in_=xg_d[:, m0:m1, :, t0 : t0 + S],
                        )
                    sweep0(H0, H1, F1, S)
                    assert (GSWEEPS + NONLY) % 2 == 0
                    bufs = [(F1, H1, H0, F0), (F0, H0, H1, F1)]
                    for i in range(GSWEEPS + NONLY):
                        sweep(*bufs[i % 2], S, full=(i < GSWEEPS))
                    # final result is in H1
                    if bi + 1 < len(blocks):
                        for s in range(BPC):
                            nc.vector.tensor_copy(hcar[:, :, s], H1[:, :, s, S])

                    # ---------- Phase C: L2 normalize + store ----------
                    for s in range(BPC):
                        pss = pcs.tile([1, SB], f32, tag="pss")
                        for j in range(HC):
                            sq = pbg.tile([128, SB], bf16, tag="sq")
                            nc.vector.tensor_mul(
                                sq[:, :S], H1[:, j, s, 1 : S + 1],
                                H1[:, j, s, 1 : S + 1],
                            )
                            nc.tensor.matmul(
                                pss[:, :S], ones_k, sq[:, :S],
                                start=(j == 0), stop=(j == HC - 1),
                            )
                        nrm = pbg.tile([1, SB], f32, tag="nrm")
                        nc.scalar.activation(nrm[:, :S], pss[:, :S], AF.Sqrt)
                        nc.vector.tensor_scalar_max(nrm[:, :S], nrm[:, :S], EPS)
                        rs = pbg.tile([1, SB], f32, tag="rs")
                        nc.vector.reciprocal(rs[:, :S], nrm[:, :S])
                        psb = pcb.tile([128, SB], f32, tag="psb")
                        nc.tensor.matmul(
                            psb[:, :S], ones_m, rs[:, :S], start=True, stop=True
                        )
                        for j in range(HC):
                            ysc = pbo.tile([128, SB], f32, tag="ysc")
                            nc.vector.tensor_mul(
                                ysc[:, :S], H1[:, j, s, 1 : S + 1], psb[:, :S]
                            )
                            nc.sync.dma_start(
                                out=yout[j, :, s * t2 + t0 : s * t2 + t0 + S],
                                in_=ysc[:, :S],
                            )
                    t0 += S

    nc.compile()
    return nc


def _build_noop():
    """Same I/O signature as _build but a trivial body — used by test.py to
    subtract dispatch/transfer overhead from wall-clock timing."""
    import concourse.mybir as mybir
    import concourse.tile as tile
    from concourse import bacc

    f32 = mybir.dt.float32
    bf16 = mybir.dt.bfloat16
    fp8 = mybir.dt.float8e4
    t2 = _t2_cached[0]
    nc = bacc.Bacc("TRN2", enable_partition_id=False)
    nc.dram_tensor("xT", [KC, 128, BPC * t2], bf16, kind="ExternalInput")
    nc.dram_tensor("wihT", [KC, 128, G3], bf16, kind="ExternalInput")
    nc.dram_tensor("whhT", [KC, 128, G3], fp8, kind="ExternalInput")
    bih = nc.dram_tensor("bih", [128, MC], f32, kind="ExternalInput")
    nc.dram_tensor("bhh", [128, MC], f32, kind="ExternalInput")
    nc.dram_tensor("eye", [128, 128], bf16, kind="ExternalInput")
    nc.dram_tensor("bhhT", [1, G3], bf16, kind="ExternalInput")
    yout = nc.dram_tensor("yout", [HC, 128, BPC * t2], f32, kind="ExternalOutput")
    with tile.TileContext(nc) as tc:
        with tc.tile_pool(name="p", bufs=1) as p:
            t = p.tile([128, MC], f32, tag="t")
            nc.sync.dma_start(out=t, in_=bih[:, :])
            nc.sync.dma_start(out=yout[0, :, :MC], in_=t)
    nc.compile()
    return nc


_t2_cached = [2048]


def _prep_inputs(x, w_ih, w_hh, b_ih, b_hh, t2):
    """Host-side layout prep (not timed): transposes + dtype casts."""
    bf = ml_dtypes.bfloat16
    f8 = ml_dtypes.float8_e4m3
    x = np.asarray(x, dtype=np.float32)[:, :t2]
    wihT = np.ascontiguousarray(np.asarray(w_ih, np.float32).T).astype(bf)
    whhT = np.ascontiguousarray(np.asarray(w_hh, np.float32).T).astype(f8)
    wihT = wihT.reshape(KC, 128, G3)
    whhT = whhT.reshape(KC, 128, G3)
    bih = np.ascontiguousarray(
        np.asarray(b_ih, np.float32).reshape(MC, 128).T
    )
    bhh = np.ascontiguousarray(
        np.asarray(b_hh, np.float32).reshape(MC, 128).T
    )
    eye = np.eye(128, dtype=bf)
    bhhT = np.asarray(b_hh, np.float32).reshape(1, G3).astype(bf)
    in_maps = []
    for c in range(NCORES):
        xc = x[c * BPC : (c + 1) * BPC]            # [2, t2, D]
        xTc = np.ascontiguousarray(xc.transpose(2, 0, 1))  # [D, 2, t2]
        xTc = xTc.reshape(KC, 128, BPC * t2).astype(bf)
        in_maps.append(
            {"xT": xTc, "wihT": wihT, "whhT": whhT, "bih": bih, "bhh": bhh,
             "eye": eye, "bhhT": bhhT}
        )
    return in_maps


def _assemble(results, lengths, t2):
    """Per-core yout [HC,128,BPC*t2] fp32 -> flat [sum(lengths), D]."""
    lengths = np.asarray(lengths).astype(np.int64)
    parts = []
    for c in range(NCORES):
        yo = np.asarray(results[c]["yout"], np.float32)
        yo = yo.reshape(D, BPC, t2).transpose(1, 2, 0)  # [2, t2, D]
        for b in range(BPC):
            parts.append(yo[b, : lengths[c * BPC + b]])
    return np.concatenate(parts, axis=0)


def kernel(x, lengths, w_ih, w_hh, b_ih, b_hh):
    from concourse import bass_utils

    lengths_np = np.asarray(lengths).astype(np.int64)
    max_len = int(lengths_np.max())
    t2 = min(T, -(-max_len // 16) * 16)
    _t2_cached[0] = t2
    if t2 not in _cache:
        _cache[t2] = _build(t2)
    nc = _cache[t2]

    in_maps = _prep_inputs(x, w_ih, w_hh, b_ih, b_hh, t2)
    res = bass_utils.run_bass_kernel_spmd(nc, in_maps, list(range(NCORES)))
    return _assemble(res.results, lengths_np, t2)


if __name__ == "__main__":
    import reference

    inputs = reference.setup_inputs()
    out = kernel(**{k: np.asarray(v) for k, v in inputs.items()})
    exp = np.asarray(reference.reference(**inputs))
    err = np.abs(out - exp).max()
    rel = np.linalg.norm(out - exp) / np.linalg.norm(exp)
    print("absmax:", err, "rel:", rel)
